# revision 18
# baseline (speedup 1.0000x reference)
"""FFQLinear Trainium2 kernel (8 NeuronCores, column-parallel).

Computes out = x2d @ W + bias with W = (q_int - zero_point) * scale, where
scale / zero_point broadcast over the OUTPUT-column axis of the [D, D] code
matrix (so W[:, j] = (q[:, j] - zp[j]) * scale[j]).

Math used on device (zp is zero in this problem; a host-side exact rank-1
correction handles the general case): since scale is per-output-column,
    out[:, j] = (x2d @ q)[:, j] * scale[j] + bias[j].

q's values are integers in [0, 256), EXACTLY representable in fp16/bf16, so
Y = x @ q runs on the PE in 16-bit with fp32 PSUM accumulation and the only
error source is x's 16-bit rounding. fp16 (10 mantissa bits) gives
rel err ~2e-4 in one pass at full PE rate. SPLIT=2 additionally splits
x == x_hi + x_lo (both 16-bit, exact sum) for ~fp32 accuracy at 2x the
matmul cost.

Sharding: column-parallel per the hint. Each of the 8 cores gets
  - x pre-transposed and pre-tiled on the host (contraction dim on SBUF
    partitions, 4KB contiguous per-partition DMA lines), replicated
  - a [K, 512] column shard of q, and [512] shards of scale/bias
and produces a [M, 512] f32 output shard. Host concatenates the shards.
"""

import sys
import time
import types

import numpy as np
import ml_dtypes

import concourse.bass as bass
import concourse.bacc as bacc
import concourse.mybir as mybir
import concourse.tile as tile

# bass_utils' axon trace path does an unguarded
# `from antenv.axon_hooks import get_axon_ntff_profile_hook`; some images
# lack that module. Provide a stub (hook=None -> tracing degrades
# gracefully) so a BASS_TRACE=1 environment can't crash the kernel.
try:
    import antenv.axon_hooks  # noqa: F401
except Exception:
    try:
        import antenv

        _stub = types.ModuleType("antenv.axon_hooks")
        _stub._HOOK = None
        _stub.set_axon_ntff_profile_hook = lambda h: setattr(_stub, "_HOOK", h)
        _stub.get_axon_ntff_profile_hook = lambda: _stub._HOOK
        sys.modules["antenv.axon_hooks"] = _stub
        antenv.axon_hooks = _stub
    except Exception:
        pass

from concourse.bass_utils import run_bass_kernel_spmd

B, S, D = 2, 2048, 4096
M = B * S            # 4096 output rows
K = D                # 4096 contraction
N = D                # 4096 output cols
NCORES = 8
NS = N // NCORES     # 512 output cols per core

P = 128
KO = K // P          # 32 k-tiles
M_CHUNK = 512        # rows per chunk (4 psum tiles of 128)
MT = M_CHUNK // P    # 4
NMC = M // M_CHUNK   # 8 m-chunks
KO_PER_DMA = 4       # k-tiles per x DMA (512KB fp16 per transfer)
NKD = KO // KO_PER_DMA  # 8 k-dma groups

SPLIT = 1            # 1 = single 16-bit pass, 2 = hi/lo split (~fp32 exact)
DT16 = "fp16"        # "bf16" or "fp16" — PE input dtype for x and q

F32 = mybir.dt.float32

_CACHE: dict = {}


def _dt16(name: str):
    return mybir.dt.float16 if name == "fp16" else mybir.dt.bfloat16


def _np16(name: str):
    return np.float16 if name == "fp16" else ml_dtypes.bfloat16


def _build(split: int, dt16_name: str) -> bass.Bass:
    # Bacc (not plain Bass): its compile() runs generate_event_semaphores,
    # which splits multi-wait DMAs to satisfy the 1-wait HW encoding limit.
    nc = bacc.Bacc(
        "TRN2", target_bir_lowering=False, debug=False, num_devices=NCORES
    )
    DT = _dt16(dt16_name)
    # Host-pretiled layouts: every DMA below reads a fully-contiguous
    # [P, KO_PER_DMA, *] block (4KB per-partition lines).
    xt = [
        nc.dram_tensor(
            f"xt{i}", [NMC * NKD, P, KO_PER_DMA, M_CHUNK], DT,
            kind="ExternalInput",
        )
        for i in range(split)
    ]
    qs = nc.dram_tensor(
        "qs", [NKD, P, KO_PER_DMA, NS], DT, kind="ExternalInput"
    )
    scale_d = nc.dram_tensor("scale", [NS], F32, kind="ExternalInput")
    bias_d = nc.dram_tensor("bias", [NS], F32, kind="ExternalInput")
    out_d = nc.dram_tensor("out", [M, NS], F32, kind="ExternalOutput")

    with tile.TileContext(nc) as tc:
        with (
            tc.tile_pool(name="const", bufs=1) as cpool,
            tc.tile_pool(name="xload", bufs=10) as xpool,
            tc.tile_pool(name="opool", bufs=4) as opool,
            tc.tile_pool(name="psum", bufs=8, space="PSUM") as ppool,
        ):
            # Resident q shard, one tile per k-dma group. The DMA for
            # chunk kd is emitted interleaved with the first m-chunk's x
            # loads so the first matmuls only wait for ~1MB, not all of q.
            qk = [
                cpool.tile([P, KO_PER_DMA, NS], DT, name=f"qk{kd}")
                for kd in range(NKD)
            ]
            scale_sb = cpool.tile([P, NS], F32)
            bias_sb = cpool.tile([P, NS], F32)

            for mc in range(NMC):
                psums = [
                    ppool.tile([P, NS], F32, name=f"ps{mt}", tag="ps")
                    for mt in range(MT)
                ]
                last_mc = mc == NMC - 1
                xtiles = []
                for kd in range(NKD):
                    if mc == 0:
                        nc.sync.dma_start(qk[kd][:], qs[kd])
                    xts = []
                    for s in range(split):
                        x_sb = xpool.tile(
                            [P, KO_PER_DMA, M_CHUNK], DT,
                            name=f"x{s}sb", tag=f"x{s}",
                        )
                        nc.sync.dma_start(x_sb[:], xt[s][mc * NKD + kd])
                        xts.append(x_sb)
                    xtiles.append(xts)
                    if last_mc:
                        continue
                    for kk in range(KO_PER_DMA):
                        ko = kd * KO_PER_DMA + kk
                        for mt in range(MT):
                            for s in range(split):
                                nc.tensor.matmul(
                                    psums[mt][:],
                                    lhsT=xts[s][:, kk, mt * P:(mt + 1) * P],
                                    rhs=qk[kd][:, kk, :],
                                    start=(ko == 0 and s == 0),
                                    stop=(ko == KO - 1 and s == split - 1),
                                )
                if last_mc:
                    # mt-major: each psum finishes (and drains through the
                    # epilogue) while later mt groups still compute, so only
                    # one tile's epilogue trails the final matmul.
                    for mt in range(MT):
                        for kd in range(NKD):
                            for kk in range(KO_PER_DMA):
                                ko = kd * KO_PER_DMA + kk
                                for s in range(split):
                                    nc.tensor.matmul(
                                        psums[mt][:],
                                        lhsT=xtiles[kd][s][:, kk, mt * P:(mt + 1) * P],
                                        rhs=qk[kd][:, kk, :],
                                        start=(ko == 0 and s == 0),
                                        stop=(ko == KO - 1 and s == split - 1),
                                    )
                        o_sb = opool.tile([P, NS], F32, name="osb", tag="o")
                        nc.vector.tensor_mul(o_sb[:], psums[mt][:], scale_sb[:])
                        nc.vector.tensor_add(o_sb[:], o_sb[:], bias_sb[:])
                        row = (mc * MT + mt) * P
                        nc.sync.dma_start(out_d[row:row + P, :], o_sb[:])
                    continue
                if mc == 0:
                    nc.sync.dma_start(
                        scale_sb[:], scale_d[None, :].to_broadcast((P, NS))
                    )
                    nc.sync.dma_start(
                        bias_sb[:], bias_d[None, :].to_broadcast((P, NS))
                    )
                for mt in range(MT):
                    o_sb = opool.tile([P, NS], F32, name="osb", tag="o")
                    nc.vector.tensor_mul(o_sb[:], psums[mt][:], scale_sb[:])
                    nc.vector.tensor_add(o_sb[:], o_sb[:], bias_sb[:])
                    row = (mc * MT + mt) * P
                    nc.sync.dma_start(out_d[row:row + P, :], o_sb[:])
    nc.compile()
    return nc


def _get_nc(split: int, dt16_name: str) -> bass.Bass:
    key = (split, dt16_name)
    if key not in _CACHE:
        _CACHE[key] = _build(split, dt16_name)
    return _CACHE[key]


def _pretile_x(x16: np.ndarray) -> np.ndarray:
    """[M, K] 16-bit -> [NMC*NKD, P, KO_PER_DMA, M_CHUNK] with
    XD[mc*NKD+kd, p, kk, m] = x16[mc*M_CHUNK + m, (kd*KO_PER_DMA+kk)*P + p]."""
    v = x16.reshape(NMC, M_CHUNK, NKD, KO_PER_DMA, P)
    v = v.transpose(0, 2, 4, 3, 1)  # (mc, kd, p, kk, m)
    return np.ascontiguousarray(v).reshape(NMC * NKD, P, KO_PER_DMA, M_CHUNK)


def _pretile_q(q16: np.ndarray) -> np.ndarray:
    """[K, NS] 16-bit -> [NKD, P, KO_PER_DMA, NS] with
    QD[kd, p, kk, n] = q16[(kd*KO_PER_DMA+kk)*P + p, n]."""
    v = q16.reshape(NKD, KO_PER_DMA, P, NS)
    return np.ascontiguousarray(v.transpose(0, 2, 1, 3))


def _prep_in_maps(x, q_int, scale, bias, split, dt16_name):
    np16 = _np16(dt16_name)
    x2d = np.ascontiguousarray(x.reshape(M, K)).astype(np.float32, copy=False)
    xt_list = []
    if split == 1:
        xt_list.append(_pretile_x(x2d.astype(np16)))
    else:
        x_hi = x2d.astype(np16)
        x_lo = (x2d - x_hi.astype(np.float32)).astype(np16)
        xt_list.append(_pretile_x(x_hi))
        xt_list.append(_pretile_x(x_lo))

    q16 = q_int.astype(np16)            # exact: values in [0, 256)
    scale_f = scale.astype(np.float32, copy=False)
    bias_f = bias.astype(np.float32, copy=False)

    in_maps = []
    for c in range(NCORES):
        m = {f"xt{i}": xt_list[i] for i in range(split)}
        m["qs"] = _pretile_q(q16[:, c * NS:(c + 1) * NS])
        m["scale"] = np.ascontiguousarray(scale_f[c * NS:(c + 1) * NS])
        m["bias"] = np.ascontiguousarray(bias_f[c * NS:(c + 1) * NS])
        in_maps.append(m)
    return in_maps


def _run(x, q_int, scale, zero_point, bias, split, dt16_name=None,
         trace=False, **trace_kw):
    dt16_name = dt16_name or DT16
    nc = _get_nc(split, dt16_name)
    in_maps = _prep_in_maps(x, q_int, scale, bias, split, dt16_name)
    res = run_bass_kernel_spmd(
        nc, in_maps, list(range(NCORES)), trace=trace, **trace_kw
    )
    out2d = np.concatenate([r["out"] for r in res.results], axis=1)

    if np.any(np.asarray(zero_point) != 0):
        # exact rank-1 correction: -= rowsum(x) ⊗ (scale * zp)
        x2d = x.reshape(M, K).astype(np.float32, copy=False)
        out2d = out2d - np.outer(
            x2d.sum(axis=1),
            scale.astype(np.float32) * zero_point.astype(np.float32),
        )

    return out2d.reshape(B, S, D).astype(np.float32, copy=False), res


def _run_subprocess(x, q_int, scale, zero_point, bias):
    """Fresh-process retry: a NRT_EXEC_UNIT_UNRECOVERABLE poisons the
    in-process PJRT client, but a new process recovers."""
    import os
    import subprocess
    import tempfile

    d = tempfile.mkdtemp(prefix="ffq_retry_")
    names = ["x", "q_int", "scale", "zero_point", "bias"]
    for name, arr in zip(names, [x, q_int, scale, zero_point, bias]):
        np.save(os.path.join(d, name + ".npy"), np.asarray(arr))
    kdir = os.path.dirname(os.path.abspath(__file__))
    code = (
        "import sys, numpy as np\n"
        f"sys.path.insert(0, {kdir!r})\n"
        "import kernel as km\n"
        f"d = {d!r}\n"
        "ins = [np.load(d + '/' + n + '.npy') for n in "
        "['x', 'q_int', 'scale', 'zero_point', 'bias']]\n"
        "out, _ = km._run(*ins, km.SPLIT)\n"
        "np.save(d + '/out.npy', out)\n"
    )
    subprocess.run([sys.executable, "-c", code], check=True, timeout=2400)
    return np.load(os.path.join(d, "out.npy"))


def kernel(x, q_int, scale, zero_point, bias):
    try:
        out, _ = _run(x, q_int, scale, zero_point, bias, SPLIT)
    except Exception:
        # transient device errors (e.g. a core wedged by a previous
        # profiling session): retry in-process, then in a fresh process
        time.sleep(5)
        try:
            out, _ = _run(x, q_int, scale, zero_point, bias, SPLIT)
        except Exception:
            out = _run_subprocess(x, q_int, scale, zero_point, bias)
    return out
